# revision 27
# baseline (speedup 1.0000x reference)
"""Trainium2 Bass kernel: per-edge gathered linear + bias + ReLU (GNN message op).

Reference computation:
    y[e] = relu(W[idx[e]] @ x[e] + b[idx[e]])
      x:   [50000, 128, 1] f32   (edge features)
      idx: [50000] int32         (pool index per edge, 0..9999)
      W:   [10000, 64, 128] f32  (weight pool)
      b:   [10000, 64, 1] f32    (bias pool)
      y:   [50000, 64, 1] f32

Strategy (host does all data-dependent layout; the device program is uniform
across cores and input values):
  1. Sort edges by pool index; split the sorted order into 8 chunks at GROUP
     boundaries with equal group counts per core (edge counts then vary ~1%).
     Slots cost 32KB of weight stream each and number ceil(groups/2), so this
     minimizes the padded max slot count, and no pool entry ships to two
     cores: pool-weight HBM traffic across the 8 cores is ~1x the pool size
     (the minimum possible).
  2. bf16 everywhere off-chip (the 2e-2 rel-err budget admits it with ~6x
     margin; fp8 weights measure right AT the budget and are rejected).
     PSUM accumulates fp32; the ReLU evacuation converts back to bf16.
  3. Variable-width slots: within a chunk, per-pool-index edge groups are
     paired two-per-slot (largest fragment with smallest, so every core's
     rank-ordered slot-width profile is nearly identical), and each slot's
     feature columns are packed tightly at the width profile's max across
     cores (~3% slack vs ~60% with fixed 16-col slots).  A slot's stationary
     operand is [128(K=in), 128(M)]: W[a].T in columns 0:64, W[b].T in
     64:128; one full-width bf16 matmul per slot (the compiler's Fast Weight
     Load kicks in) computes both groups.
  4. Slots are packed into PSUM banks (<=512 f32 columns, <=64 slots); each
     bank is one fused (weights || features) DMA, a run of matmuls, one DVE
     ReLU evacuation to bf16, one output DMA.  Input DMAs alternate between
     the two HWDGE rings (SP and ACT) so each SDMA engine round-robins both
     queues and one ring's inter-transfer gap is covered by the other's
     in-flight transfer.  ReLU runs on DVE so the DMA-issuing engines never
     block behind compute; outputs go on the SWDGE (gpsimd) queue except the
     last two banks, which use the by-then-idle HWDGE rings to shorten the
     kernel tail.
  5. Nonzero bias (not the case for this problem's inputs) falls back to the
     fixed-width 16-col-slot layout with a PSUM-seeding bias matmul.

The per-core slot count is padded to the max across cores so a single SPMD
program serves all 8 cores.
"""

import sys

for _p in (
    "/root/.axon_site",
    "/root/.axon_site/_ro/trn_rl_repo",
    "/root/.axon_site/_ro/pypackages",
    "/opt/trn_rl_repo",
    "/opt/pypackages",
):
    if _p not in sys.path:
        sys.path.append(_p)

import ml_dtypes
import numpy as np

BF16 = ml_dtypes.bfloat16

E_SEL = 50000
IN_DIM = 128
OUT_DIM = 64
N_CORES = 8
E_PER_CORE = E_SEL // N_CORES

BANK_MAX_COLS = 512                       # one PSUM bank of f32
BANK_MAX_SLOTS = 64                       # bounds the SBUF tile size
FRAG_CAP = 256                            # split larger groups (paranoia; a
                                          # pair must fit one PSUM bank)

# --- legacy fixed-width constants (bias fallback path) ---
SLOT_COLS = 16
SLOTS_PER_BANK = 32
BANK_COLS = SLOT_COLS * SLOTS_PER_BANK    # 512
W_COLS = 128
W_REGION = SLOTS_PER_BANK * W_COLS        # 4096
WX_COLS = W_REGION + BANK_COLS            # 4608


def _patch_tile_drain():
    """Split the Tile kernel-tail drain's semaphore waits across single-wait
    nops: this walrus build rejects a Drain carrying more than one sync wait
    ("Too many sync wait commands")."""
    import concourse.mybir as mybir
    import concourse.tile as tile
    from concourse.vector_clock import ScopedClock

    if getattr(tile.TileContext, "_drain_split_patch", False):
        return

    def _drain_and_barrier(self, tick_clock, wait_clock):
        nc = self.nc
        drain_inst = nc.sync.drain()
        wait_clock.add_sem_waits(
            drain_inst.ins, ScopedClock({None: tick_clock.global_clock})
        )
        si = drain_inst.ins.sync_info
        waits = list(si.on_wait) if si is not None else []
        if len(waits) > 1:
            drain_inst.ins.sync_info = mybir.SyncInfo(
                on_wait=waits[:1], on_update=list(si.on_update)
            )
            for w in waits[1:]:
                nop = nc.sync.nop(nofuse=True)
                nop.ins.sync_info = mybir.SyncInfo(on_wait=[w], on_update=[])
        nc.all_engine_barrier()
        assert self.sems is not None
        popped = nc._tile_sem_poison_stack.pop()
        assert popped is self._sem_poison
        nc.clear_and_free_semaphores(list(self.sems.allocated().values()))
        nc.all_engine_barrier()

    tile.TileContext._drain_and_barrier = _drain_and_barrier
    tile.TileContext._drain_split_patch = True


def _legalize_single_waits(nc):
    """This walrus build rejects instructions carrying more than one sync
    wait ("Too many sync wait commands").  Split every multi-wait instruction
    into single-wait nops (same engine, immediately preceding, so per-engine
    program order — and therefore the synchronization semantics — is
    preserved) followed by the original instruction with one wait."""
    import concourse.mybir as mybir

    for bb in nc.main_func.blocks:
        il = list(bb.instructions)
        new = []
        changed = False
        for ins in il:
            si = ins.sync_info
            waits = list(si.on_wait) if si is not None else []
            if len(waits) > 1:
                changed = True
                for w in waits[:-1]:
                    nop = mybir.InstNoOp(
                        name=nc.get_next_instruction_name(),
                        engine=ins.engine,
                        sync_info=mybir.SyncInfo(on_wait=[w], on_update=[]),
                        bass_nofuse=True,
                    )
                    nc.register_instruction(nop)
                    new.append(nop)
                ins.sync_info = mybir.SyncInfo(
                    on_wait=[waits[-1]], on_update=list(si.on_update)
                )
            new.append(ins)
        if changed:
            bb.instructions = new


# ---------------------------------------------------------------------------
# Variable-width path (no bias)
# ---------------------------------------------------------------------------


def _pack_core_var(idx_sorted):
    """Pair per-pool-index edge groups two-per-slot, largest fragment with
    smallest, then order slots by width descending (aligning the width
    profile across cores).

    Returns (slots, e_slot, e_col) where slots is an int64 [ns, 4] array of
    (a_val, a_n, b_val, b_n) and e_slot/e_col map each edge (in chunk-sorted
    order) to its slot rank and column within the slot (columns [0, a_n) are
    group A, [a_n, a_n + b_n) group B)."""
    vals, counts = np.unique(idx_sorted, return_counts=True)
    start_of = np.concatenate([[0], np.cumsum(counts)[:-1]])
    cursor = {int(v): int(s) for v, s in zip(vals, start_of)}
    frags = []
    for v, cnt in zip(vals, counts):
        c = int(cnt)
        while c > FRAG_CAP:
            frags.append((int(v), FRAG_CAP))
            c -= FRAG_CAP
        if c:
            frags.append((int(v), c))
    frags.sort(key=lambda t: -t[1])
    lo, hi = 0, len(frags) - 1
    slots = []
    while lo < hi:
        slots.append([frags[lo][0], frags[lo][1], frags[hi][0], frags[hi][1]])
        lo += 1
        hi -= 1
    if lo == hi:
        slots.append([frags[lo][0], frags[lo][1], -1, 0])
    slots.sort(key=lambda s: -(s[1] + s[3]))

    n_edges = len(idx_sorted)
    e_slot = np.empty(n_edges, np.int64)
    e_col = np.empty(n_edges, np.int64)
    for j, (av, an, bv, bn) in enumerate(slots):
        p = cursor[av]
        cursor[av] = p + an
        e_slot[p : p + an] = j
        e_col[p : p + an] = np.arange(an)
        if bn:
            p = cursor[bv]
            cursor[bv] = p + bn
            e_slot[p : p + bn] = j
            e_col[p : p + bn] = an + np.arange(bn)
    return np.asarray(slots, np.int64), e_slot, e_col


def _plan_banks(prof):
    """Greedy-fill PSUM banks over the (descending) shared width profile,
    then reverse so the smallest bank runs first (fast pipeline ramp).

    Returns a list of dicts with rank range, geometry, and flat offsets into
    the fused wx stream and the out stream."""
    banks = []
    start = 0
    cur_c = 0
    cur_n = 0
    for j, w in enumerate(prof.tolist()):
        # The first partition (largest-width ranks) becomes the LAST program
        # bank after the reverse below — cap it small so the kernel tail
        # (last compute + evacuation + store) is short.  The final partition
        # (leftover small-width ranks) becomes the FIRST program bank, giving
        # the DMA pipeline a fast ramp.
        cap = BANK_MAX_COLS // 2 if not banks else BANK_MAX_COLS
        if cur_c + w > cap or cur_n >= BANK_MAX_SLOTS:
            banks.append((start, j))
            start = j
            cur_c = 0
            cur_n = 0
        cur_c += w
        cur_n += 1
    banks.append((start, len(prof)))
    banks.reverse()

    geo = []
    woff = 0
    coff = 0
    for s0, s1 in banks:
        widths = prof[s0:s1]
        ns = s1 - s0
        cols = int(widths.sum())
        coloffs = np.concatenate([[0], np.cumsum(widths)[:-1]])
        L = ns * 128 + cols
        geo.append(
            dict(s0=s0, s1=s1, ns=ns, cols=cols, L=L, woff=woff, coff=coff,
                 coloffs=coloffs, widths=widths)
        )
        woff += L
        coff += cols
    return geo, woff, coff


def _build_core_inputs_var(x_chunk, slots, e_slot, e_col, Wpool, ns_max, geo,
                           total_L, rank_fbase, rank_gcol):
    a_val = np.full(ns_max, -1, np.int64)
    b_val = np.full(ns_max, -1, np.int64)
    ns = len(slots)
    if ns:
        a_val[:ns] = slots[:, 0]
        b_val[:ns] = slots[:, 2]

    wx = np.zeros((128, total_L), np.float32)
    for g in geo:
        av = a_val[g["s0"] : g["s1"]]
        bv = b_val[g["s0"] : g["s1"]]
        A = np.zeros((g["ns"], 128, 128), np.float32)
        ma = av >= 0
        A[ma, :, :OUT_DIM] = Wpool[av[ma]].transpose(0, 2, 1)
        mb = bv >= 0
        A[mb, :, OUT_DIM:] = Wpool[bv[mb]].transpose(0, 2, 1)
        wx[:, g["woff"] : g["woff"] + g["ns"] * 128] = A.transpose(1, 0, 2).reshape(
            128, -1
        )
    fcol = rank_fbase[e_slot] + e_col
    wx[:, fcol] = x_chunk.T
    gcol = rank_gcol[e_slot] + e_col
    return {"wx": wx.astype(BF16)}, gcol


def _build_program_var(geo, total_L, total_cols):
    from contextlib import ExitStack

    import concourse.bass as bass
    import concourse.mybir as mybir
    import concourse.tile as tile

    _patch_tile_drain()
    f32 = mybir.dt.float32
    bf16 = mybir.dt.bfloat16

    nc = bass.Bass()
    wx = nc.declare_dram_parameter("wx", [128, total_L], bf16, isOutput=False)
    out = nc.declare_dram_parameter("out", [128, total_cols], bf16, isOutput=True)

    with ExitStack() as ctx:
        tc = ctx.enter_context(tile.TileContext(nc))
        wxp = ctx.enter_context(tc.tile_pool(name="wx", bufs=6))
        # Deep out pool ([128, <=512] bf16 is only ~1KB/partition): the ReLU
        # practically never waits on an output-DMA completion to recycle a
        # buffer, so the evacuation pipeline can't back up near the tail.
        op = ctx.enter_context(tc.tile_pool(name="o", bufs=min(8, len(geo))))
        pp = ctx.enter_context(tc.tile_pool(name="ps", bufs=6, space="PSUM"))
        nb = len(geo)
        for b, g in enumerate(geo):
            wxt = wxp.tile([128, g["L"]], bf16)
            (nc.sync if b % 2 == 0 else nc.scalar).dma_start(
                wxt[:], wx[:, g["woff"] : g["woff"] + g["L"]]
            )
            ps = pp.tile([128, g["cols"]], f32)
            FB = g["ns"] * 128
            for k in range(g["ns"]):
                co = int(g["coloffs"][k])
                w = int(g["widths"][k])
                nc.tensor.matmul(
                    ps[:, co : co + w],
                    wxt[:, k * 128 : (k + 1) * 128],
                    wxt[:, FB + co : FB + co + w],
                    start=True,
                    stop=True,
                )
            ot = op.tile([128, g["cols"]], bf16)
            # DVE evacuation keeps the DMA-issuing engines (SP/ACT) free.
            nc.vector.tensor_relu(ot[:], ps[:])
            oslice = out[:, g["coff"] : g["coff"] + g["cols"]]
            if b >= nb - 2:
                # Tail banks: HWDGE beats SWDGE's completion latency.  Use
                # the SAME ring as this bank's input — it is emitted after
                # that input and no later input queues behind it on this
                # engine, so its relu-wait cannot stall the input stream.
                (nc.sync if b % 2 == 0 else nc.scalar).dma_start(oslice, ot[:])
            else:
                nc.gpsimd.dma_start(oslice, ot[:])
    _legalize_single_waits(nc)
    return nc


def _prepare_var(x, idx, W):
    perm = np.argsort(idx, kind="stable")
    # Chunk at GROUP boundaries with ~equal group counts per core (not equal
    # edge counts): the per-core slot count is ceil(groups/2), so this
    # minimizes the padded max slot count (the weight stream is 32KB/slot),
    # and no group straddles two cores (no duplicated pool entries).  Edge
    # counts then vary only ~1% core-to-core.
    _, counts = np.unique(idx[perm], return_counts=True)
    g_edges = np.concatenate([[0], np.cumsum(counts)])
    n_groups = len(counts)
    packs = []
    for c in range(N_CORES):
        g0 = round(c * n_groups / N_CORES)
        g1 = round((c + 1) * n_groups / N_CORES)
        chunk = perm[g_edges[g0] : g_edges[g1]]
        slots, e_slot, e_col = _pack_core_var(idx[chunk])
        packs.append((chunk, slots, e_slot, e_col))

    ns_max = max(len(p[1]) for p in packs)
    widths = np.zeros((N_CORES, ns_max), np.int64)
    for c, (_, slots, _, _) in enumerate(packs):
        if len(slots):
            widths[c, : len(slots)] = slots[:, 1] + slots[:, 3]
    prof = widths.max(axis=0)
    geo, total_L, total_cols = _plan_banks(prof)

    rank_fbase = np.empty(ns_max, np.int64)
    rank_gcol = np.empty(ns_max, np.int64)
    for g in geo:
        rank_fbase[g["s0"] : g["s1"]] = g["woff"] + g["ns"] * 128 + g["coloffs"]
        rank_gcol[g["s0"] : g["s1"]] = g["coff"] + g["coloffs"]

    in_maps = []
    scatter = []
    for chunk, slots, e_slot, e_col in packs:
        core_in, gcol = _build_core_inputs_var(
            x[chunk], slots, e_slot, e_col, W, ns_max, geo, total_L,
            rank_fbase, rank_gcol,
        )
        in_maps.append(core_in)
        half = (e_col >= slots[e_slot, 1]).astype(np.int64)
        scatter.append((chunk, gcol, half))
    return in_maps, geo, total_L, total_cols, scatter


def _unshard_var(results, scatter):
    y_full = np.empty((E_SEL, OUT_DIM), np.float32)
    for c, (chunk, gcol, half) in enumerate(scatter):
        o = results[c]["out"].astype(np.float32)
        halves = o.reshape(2, OUT_DIM, -1)
        y_full[chunk] = halves[half, :, gcol]
    return y_full.reshape(E_SEL, OUT_DIM, 1)


# ---------------------------------------------------------------------------
# Legacy fixed-width path (bias fallback)
# ---------------------------------------------------------------------------


def _pack_chunk(idx_sorted):
    """Pack one core's sorted pool indices into fixed 16-col slots."""
    vals, counts = np.unique(idx_sorted, return_counts=True)
    n_edges = len(idx_sorted)
    nruns = len(vals)
    slot_a, slot_b = [], []
    edge_slot = np.empty(n_edges, np.int64)
    edge_col = np.empty(n_edges, np.int64)
    edge_half = np.empty(n_edges, np.int64)
    i = 0
    rem = int(counts[0]) if nruns else 0
    pos = 0
    while i < nruns:
        s = len(slot_a)
        a = int(vals[i])
        take_a = min(rem, SLOT_COLS)
        edge_slot[pos : pos + take_a] = s
        edge_col[pos : pos + take_a] = np.arange(take_a)
        edge_half[pos : pos + take_a] = 0
        pos += take_a
        rem -= take_a
        if rem == 0:
            i += 1
            rem = int(counts[i]) if i < nruns else 0
        b = -1
        if take_a < SLOT_COLS and i < nruns:
            b = int(vals[i])
            take_b = min(rem, SLOT_COLS - take_a)
            edge_slot[pos : pos + take_b] = s
            edge_col[pos : pos + take_b] = take_a + np.arange(take_b)
            edge_half[pos : pos + take_b] = 1
            pos += take_b
            rem -= take_b
            if rem == 0:
                i += 1
                rem = int(counts[i]) if i < nruns else 0
        slot_a.append(a)
        slot_b.append(b)
    assert pos == n_edges
    return (
        np.asarray(slot_a, np.int64),
        np.asarray(slot_b, np.int64),
        edge_slot,
        edge_col,
        edge_half,
    )


def _build_core_inputs(x_chunk, a_arr, b_arr, edge_slot, edge_col, W, B,
                       n_slots_pad):
    n_tiles = n_slots_pad // SLOTS_PER_BANK
    n_slots = len(a_arr)

    lhsT = np.zeros((n_slots_pad, 128, 128), np.float32)
    mask_a = a_arr >= 0
    lhsT[:n_slots][mask_a, :, :OUT_DIM] = W[a_arr[mask_a]].transpose(0, 2, 1)
    mask_b = b_arr >= 0
    lhsT[:n_slots][mask_b, :, OUT_DIM:] = W[b_arr[mask_b]].transpose(0, 2, 1)

    xcols = np.zeros((IN_DIM, n_slots_pad * SLOT_COLS), np.float32)
    gcol = edge_slot * SLOT_COLS + edge_col
    xcols[:, gcol] = x_chunk.T

    wx = np.empty((n_tiles, 128, WX_COLS), BF16)
    wx[:, :, :W_REGION] = (
        lhsT.reshape(n_tiles, SLOTS_PER_BANK, 128, 128)
        .transpose(0, 2, 1, 3)
        .reshape(n_tiles, 128, W_REGION)
    )
    wx[:, :, W_REGION:] = xcols.reshape(128, n_tiles, BANK_COLS).transpose(1, 0, 2)

    core_in = {"wx": wx}
    bm = np.zeros((n_slots_pad, 128), np.float32)
    bm[:n_slots][mask_a, :OUT_DIM] = B[a_arr[mask_a]]
    bm[:n_slots][mask_b, OUT_DIM:] = B[b_arr[mask_b]]
    core_in["biasslab"] = bm.reshape(n_tiles, SLOTS_PER_BANK, 128).astype(BF16)
    ind = np.zeros((SLOTS_PER_BANK, BANK_COLS), np.float32)
    for s in range(SLOTS_PER_BANK):
        ind[s, s * SLOT_COLS : (s + 1) * SLOT_COLS] = 1.0
    core_in["ind"] = ind.astype(BF16)
    return core_in


def _build_program_bias(n_tiles):
    from contextlib import ExitStack

    import concourse.bass as bass
    import concourse.mybir as mybir
    import concourse.tile as tile

    _patch_tile_drain()
    f32 = mybir.dt.float32
    bf16 = mybir.dt.bfloat16

    nc = bass.Bass()
    wx = nc.declare_dram_parameter("wx", [n_tiles, 128, WX_COLS], bf16, isOutput=False)
    bsl = nc.declare_dram_parameter(
        "biasslab", [n_tiles, SLOTS_PER_BANK, 128], bf16, isOutput=False
    )
    ind = nc.declare_dram_parameter(
        "ind", [SLOTS_PER_BANK, BANK_COLS], bf16, isOutput=False
    )
    out = nc.declare_dram_parameter(
        "out", [n_tiles, 128, BANK_COLS], bf16, isOutput=True
    )

    with ExitStack() as ctx:
        tc = ctx.enter_context(tile.TileContext(nc))
        wxp = ctx.enter_context(tc.tile_pool(name="wx", bufs=3))
        op = ctx.enter_context(tc.tile_pool(name="o", bufs=3))
        pp = ctx.enter_context(tc.tile_pool(name="ps", bufs=4, space="PSUM"))
        cp = ctx.enter_context(tc.tile_pool(name="const", bufs=1))
        bp = ctx.enter_context(tc.tile_pool(name="b", bufs=3))
        ind_t = cp.tile([128, BANK_COLS], bf16)
        nc.sync.dma_start(ind_t[:SLOTS_PER_BANK, :], ind[:, :])
        for t in range(n_tiles):
            wxt = wxp.tile([128, WX_COLS], bf16)
            nc.sync.dma_start(wxt[:], wx[t])
            ps = pp.tile([128, BANK_COLS], f32)
            bt = bp.tile([128, 128], bf16)
            nc.sync.dma_start(bt[:SLOTS_PER_BANK, :], bsl[t])
            nc.tensor.matmul(
                ps[:],
                bt[:SLOTS_PER_BANK, :],
                ind_t[:SLOTS_PER_BANK, :],
                start=True,
                stop=False,
                skip_group_check=True,
            )
            for s in range(SLOTS_PER_BANK):
                rhs = wxt[:, W_REGION + s * SLOT_COLS : W_REGION + (s + 1) * SLOT_COLS]
                cs = slice(s * SLOT_COLS, (s + 1) * SLOT_COLS)
                nc.tensor.matmul(
                    ps[:, cs],
                    wxt[:, s * W_COLS : (s + 1) * W_COLS],
                    rhs,
                    start=False,
                    stop=True,
                    skip_group_check=True,
                )
            ot = op.tile([128, BANK_COLS], bf16)
            nc.scalar.activation(ot[:], ps[:], mybir.ActivationFunctionType.Relu)
            nc.scalar.dma_start(out[t], ot[:])
    _legalize_single_waits(nc)
    return nc


def _prepare_bias(x, idx, W, B):
    perm = np.argsort(idx, kind="stable")
    packs = []
    for c in range(N_CORES):
        chunk = perm[c * E_PER_CORE : (c + 1) * E_PER_CORE]
        packs.append((chunk, _pack_chunk(idx[chunk])))

    max_slots = max(len(p[1][0]) for p in packs)
    n_slots_pad = -(-max_slots // SLOTS_PER_BANK) * SLOTS_PER_BANK
    n_tiles = n_slots_pad // SLOTS_PER_BANK

    in_maps = []
    scatter = []
    for chunk, (a_arr, b_arr, e_slot, e_col, e_half) in packs:
        in_maps.append(
            _build_core_inputs(x[chunk], a_arr, b_arr, e_slot, e_col, W, B,
                               n_slots_pad)
        )
        scatter.append((chunk, e_slot * SLOT_COLS + e_col, e_half))
    return in_maps, n_tiles, scatter


def _unshard_bias(results, scatter, n_tiles):
    y_full = np.empty((E_SEL, OUT_DIM), np.float32)
    for c, (chunk, gcol, half) in enumerate(scatter):
        outcols = (
            results[c]["out"]
            .astype(np.float32)
            .transpose(1, 0, 2)
            .reshape(128, n_tiles * BANK_COLS)
        )
        halves = outcols.reshape(2, OUT_DIM, n_tiles * BANK_COLS)
        y_full[chunk] = halves[half, :, gcol]
    return y_full.reshape(E_SEL, OUT_DIM, 1)


# ---------------------------------------------------------------------------


def _run(inputs, trace=False):
    from concourse.bass_utils import run_bass_kernel_spmd

    x = np.ascontiguousarray(np.asarray(inputs["nodes_features_input"], np.float32))
    x = x.reshape(E_SEL, IN_DIM)
    idx = np.asarray(inputs["edges_index"]).astype(np.int64)
    W = np.ascontiguousarray(np.asarray(inputs["edges_input_core"], np.float32))
    B = np.ascontiguousarray(
        np.asarray(inputs["edges_input_bias"], np.float32)
    ).reshape(-1, OUT_DIM)
    has_bias = bool(np.any(B))

    kw = {}
    if trace:
        kw = dict(trace=True, trace_cores=list(range(N_CORES)))

    if has_bias:
        in_maps, n_tiles, scatter = _prepare_bias(x, idx, W, B)
        nc = _build_program_bias(n_tiles)
    else:
        in_maps, geo, total_L, total_cols, scatter = _prepare_var(x, idx, W)
        nc = _build_program_var(geo, total_L, total_cols)

    try:
        res = run_bass_kernel_spmd(nc, in_maps, list(range(N_CORES)), **kw)
    except ModuleNotFoundError:
        # NTFF profiling hook unavailable in this container; run untraced.
        res = run_bass_kernel_spmd(nc, in_maps, list(range(N_CORES)))

    if has_bias:
        y = _unshard_bias(res.results, scatter, n_tiles)
    else:
        y = _unshard_var(res.results, scatter)
    return y, res.exec_time_ns


def kernel(**inputs):
    y, _ = _run(inputs, trace=False)
    return y
